# revision 6
# baseline (speedup 1.0000x reference)
"""Trainium2 Bass kernel for DeepseekV4 SWA cache gather (sparse_attention).

Contract: kernel(**inputs) takes FULL unsharded inputs, distributes across 8
NeuronCores (data-parallel over the 256 decode tokens, 32 per core; kv_cache /
seq_lens / query_start_loc / block_table replicated), runs a Bass/Tile kernel
via run_bass_kernel_spmd, and reassembles the full outputs:

    (swa_indices [256,512] i32, swa_lens [256] i32,
     prefill_gather_lens [8] i32, gathered [256,512,256] f32)

Gather strategy (general, paged): per token the 512-slot window decomposes
into <=9 runs, each contiguous inside one KV block. HW indirect DMA moves,
per partition, ONE dynamic offset + a contiguous extent. So per (token, k):
  read:    64 rows from kv[btfp[req, A+k]*64]        (block aligned; btfp is
           the block table with jax's OOB clamp pre-baked as column padding)
  mask:    rows with window pos >= swa_len zeroed in SBUF
  scatter: 64 rows to output row 64 + i*576 + 64k - r  (gutter-padded layout,
           spills from window edges land in 64-row gutters between tokens)
Host extracts the 512-row token slices. swa_indices are computed on-device
with a one-hot-over-9 select in a [128,128] layout.

Self-contained: all shapes/constants hardcoded.
"""

import numpy as np

import concourse.bass as bass
import concourse.mybir as mybir
import concourse.tile as tile
import concourse.bacc as bacc
from concourse.bass import IndirectOffsetOnAxis
from concourse.bass_utils import run_bass_kernel_spmd

# ---- problem constants (hardcoded per contract) ----
NUM_DECODES = 256
NUM_PREFILLS = 8
WINDOW = 512
BLOCK_SIZE = 64
MAX_BLOCKS = 32
HEAD = 256
NUM_REQS = NUM_DECODES + NUM_PREFILLS          # 264
NUM_SLOTS = NUM_REQS * MAX_BLOCKS * BLOCK_SIZE  # 540672

N_CORES = 8
TPC = NUM_DECODES // N_CORES                   # 32 tokens per core
P = 128
COLS = TPC * WINDOW // P                       # 128 (swa_indices layout)
NRUN = 9                                       # blocks per token window
BTW = MAX_BLOCKS + 33                          # 65: clamp-padded bt row width
STRIDE = NRUN * BLOCK_SIZE                     # 576-row per-token pitch
GUT = BLOCK_SIZE                               # 64-row head gutter
BATCHES = [(0, 12), (12, 12), (24, 8)]         # (start token, n tokens)
BIG = 1 << 20

i32 = mybir.dt.int32
f32 = mybir.dt.float32

_CACHE = {}


def _og_rows(nb):
    return GUT + nb * STRIDE


def _build():
    nc = bacc.Bacc("TRN2", target_bir_lowering=False, debug=False,
                   num_devices=N_CORES)
    A = mybir.AluOpType

    # replicated inputs
    kv = nc.dram_tensor("kv", [NUM_SLOTS, HEAD], f32, kind="ExternalInput")
    # meta2[r] = (seq_lens[r], query_start_loc[r+1]) zipped on host
    meta2 = nc.dram_tensor("meta2", [NUM_REQS * 2, 1], i32, kind="ExternalInput")
    qsl = nc.dram_tensor("qsl", [NUM_REQS + 1, 1], i32, kind="ExternalInput")
    seqlens = nc.dram_tensor("seqlens", [NUM_REQS, 1], i32, kind="ExternalInput")
    btfp = nc.dram_tensor("btfp", [NUM_REQS * BTW, 1], i32, kind="ExternalInput")
    cw = nc.dram_tensor("cw", [P, COLS], i32, kind="ExternalInput")
    cu = nc.dram_tensor("cu", [P, BLOCK_SIZE], i32, kind="ExternalInput")
    # per-core (token x4 layout for swa_indices path)
    req4 = nc.dram_tensor("req4", [P, 1], i32, kind="ExternalInput")
    sm4 = nc.dram_tensor("sm4", [P, 1], i32, kind="ExternalInput")
    ctok = nc.dram_tensor("ctok", [P, 1], i32, kind="ExternalInput")
    # per-core pair layouts, one per batch: p <-> (il, k) = (p//9, p%9)
    preq, ptok, ppk, ppdest, ppsm = [], [], [], [], []
    for b, _ in enumerate(BATCHES):
        preq.append(nc.dram_tensor(f"preq{b}", [P, 1], i32, kind="ExternalInput"))
        ptok.append(nc.dram_tensor(f"ptok{b}", [P, 1], i32, kind="ExternalInput"))
        ppk.append(nc.dram_tensor(f"ppk{b}", [P, 1], i32, kind="ExternalInput"))
        ppdest.append(nc.dram_tensor(f"ppdest{b}", [P, 1], i32,
                                     kind="ExternalInput"))
        ppsm.append(nc.dram_tensor(f"ppsm{b}", [P, 1], i32, kind="ExternalInput"))

    # outputs
    o_sidx = nc.dram_tensor("o_sidx", [P, COLS], i32, kind="ExternalOutput")
    o_slen = nc.dram_tensor("o_slen", [TPC, 1], i32, kind="ExternalOutput")
    o_pgl = nc.dram_tensor("o_pgl", [NUM_PREFILLS, 1], i32, kind="ExternalOutput")
    o_g = [nc.dram_tensor(f"o_g{b}", [_og_rows(nb), HEAD], f32,
                          kind="ExternalOutput")
           for b, (_, nb) in enumerate(BATCHES)]

    with tile.TileContext(nc) as tc:
        with tc.tile_pool(name="meta", bufs=1) as mp, \
             tc.tile_pool(name="gp", bufs=2) as gp:
            # ========== phase 0: loads ==========
            t_req4 = mp.tile([P, 1], i32)
            nc.sync.dma_start(out=t_req4[:], in_=req4[:])
            t_sm4 = mp.tile([P, 1], i32)
            nc.sync.dma_start(out=t_sm4[:], in_=sm4[:])
            t_ctok = mp.tile([P, 1], i32)
            nc.sync.dma_start(out=t_ctok[:], in_=ctok[:])
            t_cw = mp.tile([P, COLS], i32)
            nc.sync.dma_start(out=t_cw[:], in_=cw[:])
            t_cu = mp.tile([P, BLOCK_SIZE], i32)
            nc.sync.dma_start(out=t_cu[:], in_=cu[:])
            bt_req, bt_tok, bt_k, bt_dst0, bt_sm = [], [], [], [], []
            for b, _ in enumerate(BATCHES):
                x = mp.tile([P, 1], i32, tag=f"breq{b}")
                nc.sync.dma_start(out=x[:], in_=preq[b][:])
                bt_req.append(x)
                x = mp.tile([P, 1], i32, tag=f"btok{b}")
                nc.sync.dma_start(out=x[:], in_=ptok[b][:])
                bt_tok.append(x)
                x = mp.tile([P, 1], i32, tag=f"bk{b}")
                nc.sync.dma_start(out=x[:], in_=ppk[b][:])
                bt_k.append(x)
                x = mp.tile([P, 1], i32, tag=f"bdst0{b}")
                nc.sync.dma_start(out=x[:], in_=ppdest[b][:])
                bt_dst0.append(x)
                x = mp.tile([P, 1], i32, tag=f"bsm{b}")
                nc.sync.dma_start(out=x[:], in_=ppsm[b][:])
                bt_sm.append(x)

            # ========== phase 1: batch metadata (pair layout) ==========
            b_ridx, b_dst, b_mask = [], [], []
            for b, (tok0, nb) in enumerate(BATCHES):
                b_r2 = mp.tile([P, 1], i32, tag=f"br2{b}")
                nc.vector.tensor_scalar(out=b_r2[:], in0=bt_req[b][:],
                                        scalar1=2, scalar2=None, op0=A.mult)
                b_m2 = mp.tile([P, 2], i32, tag=f"bm2{b}")
                nc.gpsimd.indirect_dma_start(
                    out=b_m2[:], out_offset=None, in_=meta2[:],
                    in_offset=IndirectOffsetOnAxis(ap=b_r2[:, :1], axis=0))
                b_pos = mp.tile([P, 1], i32, tag=f"bpos{b}")
                # pos = seq - qe + tok   (qs cancels from seq - qlen - qs)
                nc.vector.tensor_tensor(out=b_pos[:], in0=b_m2[:, 0:1],
                                        in1=b_m2[:, 1:2], op=A.subtract)
                nc.vector.tensor_tensor(out=b_pos[:], in0=b_pos[:],
                                        in1=bt_tok[b][:], op=A.add)
                b_a = mp.tile([P, 1], i32, tag=f"ba{b}")
                nc.vector.tensor_scalar(out=b_a[:], in0=b_pos[:],
                                        scalar1=WINDOW - 1, scalar2=0,
                                        op0=A.subtract, op1=A.max)
                b_len = mp.tile([P, 1], i32, tag=f"blen{b}")
                nc.vector.tensor_scalar(out=b_len[:], in0=b_pos[:],
                                        scalar1=1, scalar2=WINDOW,
                                        op0=A.add, op1=A.min)
                b_vm = mp.tile([P, 1], i32, tag=f"bvm{b}")
                nc.vector.tensor_scalar(out=b_vm[:], in0=bt_sm[b][:],
                                        scalar1=0, scalar2=None, op0=A.is_ge)
                nc.vector.tensor_tensor(out=b_len[:], in0=b_len[:],
                                        in1=b_vm[:], op=A.mult)
                b_A = mp.tile([P, 1], i32, tag=f"bA{b}")
                nc.vector.tensor_scalar(out=b_A[:], in0=b_a[:],
                                        scalar1=6, scalar2=None,
                                        op0=A.arith_shift_right)
                b_r = mp.tile([P, 1], i32, tag=f"br{b}")
                nc.vector.tensor_scalar(out=b_r[:], in0=b_a[:],
                                        scalar1=BLOCK_SIZE - 1, scalar2=None,
                                        op0=A.bitwise_and)
                # btfp flat index = req*65 + A + k (clamp pre-baked in btfp)
                b_ki = mp.tile([P, 1], i32, tag=f"bki{b}")
                nc.vector.tensor_tensor(out=b_ki[:], in0=b_A[:],
                                        in1=bt_k[b][:], op=A.add)
                b_r65 = mp.tile([P, 1], i32, tag=f"br65{b}")
                nc.vector.tensor_scalar(out=b_r65[:], in0=bt_req[b][:],
                                        scalar1=BTW, scalar2=None, op0=A.mult)
                nc.vector.tensor_tensor(out=b_ki[:], in0=b_ki[:], in1=b_r65[:],
                                        op=A.add)
                b_blk = mp.tile([P, 1], i32, tag=f"bblk{b}")
                nc.gpsimd.indirect_dma_start(
                    out=b_blk[:], out_offset=None, in_=btfp[:],
                    in_offset=IndirectOffsetOnAxis(ap=b_ki[:, :1], axis=0))
                x = mp.tile([P, 1], i32, tag=f"bridx{b}")
                nc.vector.tensor_scalar(out=x[:], in0=b_blk[:],
                                        scalar1=BLOCK_SIZE, scalar2=None,
                                        op0=A.mult)
                b_ridx.append(x)
                # dest row = (64 + il*576 + 64k) - r   (pad lanes: BIG)
                x = mp.tile([P, 1], i32, tag=f"bdst{b}")
                nc.vector.tensor_tensor(out=x[:], in0=bt_dst0[b][:],
                                        in1=b_r[:], op=A.subtract)
                b_dst.append(x)
                # keep-threshold: row u kept iff u < thr = len + r - 64k
                b_thr = mp.tile([P, 1], i32, tag=f"bthr{b}")
                nc.vector.tensor_tensor(out=b_thr[:], in0=b_len[:], in1=b_r[:],
                                        op=A.add)
                b_k64 = mp.tile([P, 1], i32, tag=f"bk64{b}")
                nc.vector.tensor_scalar(out=b_k64[:], in0=bt_k[b][:],
                                        scalar1=BLOCK_SIZE, scalar2=None,
                                        op0=A.mult)
                nc.vector.tensor_tensor(out=b_thr[:], in0=b_thr[:],
                                        in1=b_k64[:], op=A.subtract)
                b_thrf = mp.tile([P, 1], f32, tag=f"bthrf{b}")
                nc.vector.tensor_copy(out=b_thrf[:], in_=b_thr[:])
                x = mp.tile([P, BLOCK_SIZE], i32, tag=f"bmask{b}")
                nc.vector.tensor_scalar(out=x[:], in0=t_cu[:],
                                        scalar1=b_thrf[:, :1], scalar2=None,
                                        op0=A.is_lt)
                b_mask.append(x)

            # ========== phase 2/3/4: read -> mask -> scatter ==========
            gts = []
            for b, (tok0, nb) in enumerate(BATCHES):
                gt = gp.tile([P, BLOCK_SIZE * HEAD], f32, tag="gblk")
                nc.gpsimd.indirect_dma_start(
                    out=gt[:], out_offset=None, in_=kv[:],
                    in_offset=IndirectOffsetOnAxis(ap=b_ridx[b][:, :1], axis=0))
                gts.append(gt)
            for b, (tok0, nb) in enumerate(BATCHES):
                g3 = gts[b][:].rearrange("p (n d) -> p n d", n=BLOCK_SIZE)
                nc.vector.tensor_tensor(
                    out=g3, in0=g3,
                    in1=b_mask[b][:].to_broadcast([P, BLOCK_SIZE, HEAD]),
                    op=A.mult)
            for b, (tok0, nb) in enumerate(BATCHES):
                nc.gpsimd.indirect_dma_start(
                    out=o_g[b][:], out_offset=IndirectOffsetOnAxis(
                        ap=b_dst[b][:, :1], axis=0),
                    in_=gts[b][:], in_offset=None,
                    bounds_check=_og_rows(nb) - 1, oob_is_err=False)

            # ========== swa_indices path (token x4 layout) ==========
            t_r2 = mp.tile([P, 1], i32)
            nc.vector.tensor_scalar(out=t_r2[:], in0=t_req4[:],
                                    scalar1=2, scalar2=None, op0=A.mult)
            t_m2 = mp.tile([P, 2], i32)
            nc.gpsimd.indirect_dma_start(
                out=t_m2[:], out_offset=None, in_=meta2[:],
                in_offset=IndirectOffsetOnAxis(ap=t_r2[:, :1], axis=0))
            t_pos = mp.tile([P, 1], i32)
            nc.vector.tensor_tensor(out=t_pos[:], in0=t_m2[:, 0:1],
                                    in1=t_m2[:, 1:2], op=A.subtract)
            nc.vector.tensor_tensor(out=t_pos[:], in0=t_pos[:], in1=t_ctok[:],
                                    op=A.add)
            t_a = mp.tile([P, 1], i32)
            nc.vector.tensor_scalar(out=t_a[:], in0=t_pos[:],
                                    scalar1=WINDOW - 1, scalar2=0,
                                    op0=A.subtract, op1=A.max)
            t_len = mp.tile([P, 1], i32)
            nc.vector.tensor_scalar(out=t_len[:], in0=t_pos[:],
                                    scalar1=1, scalar2=WINDOW,
                                    op0=A.add, op1=A.min)
            t_vm = mp.tile([P, 1], i32)
            nc.vector.tensor_scalar(out=t_vm[:], in0=t_sm4[:],
                                    scalar1=0, scalar2=None, op0=A.is_ge)
            nc.vector.tensor_tensor(out=t_len[:], in0=t_len[:], in1=t_vm[:],
                                    op=A.mult)
            t_lenf = mp.tile([P, 1], f32)
            nc.vector.tensor_copy(out=t_lenf[:], in_=t_len[:])
            t_A = mp.tile([P, 1], i32)
            nc.vector.tensor_scalar(out=t_A[:], in0=t_a[:],
                                    scalar1=6, scalar2=None,
                                    op0=A.arith_shift_right)
            t_r = mp.tile([P, 1], i32)
            nc.vector.tensor_scalar(out=t_r[:], in0=t_a[:],
                                    scalar1=BLOCK_SIZE - 1, scalar2=None,
                                    op0=A.bitwise_and)
            # bt9 = btfp[req*65 + A .. +9] in one consecutive-extent gather
            t_i9 = mp.tile([P, 1], i32)
            nc.vector.tensor_scalar(out=t_i9[:], in0=t_req4[:],
                                    scalar1=BTW, scalar2=None, op0=A.mult)
            nc.vector.tensor_tensor(out=t_i9[:], in0=t_i9[:], in1=t_A[:],
                                    op=A.add)
            t_bt9 = mp.tile([P, NRUN], i32)
            nc.gpsimd.indirect_dma_start(
                out=t_bt9[:], out_offset=None, in_=btfp[:],
                in_offset=IndirectOffsetOnAxis(ap=t_i9[:, :1], axis=0))
            t_bt9f = mp.tile([P, NRUN], f32)
            nc.vector.tensor_scalar(out=t_bt9f[:], in0=t_bt9[:],
                                    scalar1=float(BLOCK_SIZE), scalar2=None,
                                    op0=A.mult)
            # km=(r+w)>>6, lo=(a+w)&63, one-hot over k
            t_rw = mp.tile([P, COLS], i32)
            nc.vector.tensor_tensor(out=t_rw[:], in0=t_cw[:],
                                    in1=t_r[:, :1].to_broadcast([P, COLS]),
                                    op=A.add)
            t_km = mp.tile([P, COLS], i32)
            nc.vector.tensor_scalar(out=t_km[:], in0=t_rw[:],
                                    scalar1=6, scalar2=None,
                                    op0=A.arith_shift_right)
            t_aw = mp.tile([P, COLS], i32)
            nc.vector.tensor_tensor(out=t_aw[:], in0=t_cw[:],
                                    in1=t_a[:, :1].to_broadcast([P, COLS]),
                                    op=A.add)
            t_lo = mp.tile([P, COLS], i32)
            nc.vector.tensor_scalar(out=t_lo[:], in0=t_aw[:],
                                    scalar1=BLOCK_SIZE - 1, scalar2=None,
                                    op0=A.bitwise_and)
            t_acc = mp.tile([P, COLS], i32)
            nc.vector.tensor_scalar(out=t_acc[:], in0=t_km[:],
                                    scalar1=0, scalar2=t_bt9f[:, 0:1],
                                    op0=A.is_equal, op1=A.mult)
            t_term = mp.tile([P, COLS], i32)
            for k in range(1, NRUN):
                nc.vector.tensor_scalar(out=t_term[:], in0=t_km[:],
                                        scalar1=k, scalar2=t_bt9f[:, k:k + 1],
                                        op0=A.is_equal, op1=A.mult)
                nc.vector.tensor_tensor(out=t_acc[:], in0=t_acc[:],
                                        in1=t_term[:], op=A.add)
            t_slot = mp.tile([P, COLS], i32)
            nc.vector.tensor_tensor(out=t_slot[:], in0=t_acc[:], in1=t_lo[:],
                                    op=A.add)
            t_inw = mp.tile([P, COLS], i32)
            nc.vector.tensor_scalar(out=t_inw[:], in0=t_cw[:],
                                    scalar1=t_lenf[:, :1], scalar2=None,
                                    op0=A.is_lt)
            t_sidx = mp.tile([P, COLS], i32)
            nc.vector.tensor_scalar(out=t_sidx[:], in0=t_slot[:],
                                    scalar1=1, scalar2=None, op0=A.add)
            nc.vector.tensor_tensor(out=t_sidx[:], in0=t_sidx[:], in1=t_inw[:],
                                    op=A.mult)
            nc.vector.tensor_scalar(out=t_sidx[:], in0=t_sidx[:],
                                    scalar1=1, scalar2=None, op0=A.subtract)
            nc.sync.dma_start(out=o_sidx[:], in_=t_sidx[:])

            # swa_lens out: [128,1] -> [32,4] -> column 0
            t_l32 = mp.tile([TPC, 4], i32)
            nc.sync.dma_start(out=t_l32[:], in_=t_len[:])
            nc.sync.dma_start(out=o_slen[:], in_=t_l32[:, 0:1])

            # prefill_gather_lens (identical on all cores; host uses core 0)
            t_ps = mp.tile([NUM_PREFILLS, 1], i32)
            nc.sync.dma_start(out=t_ps[:], in_=seqlens[NUM_DECODES:NUM_REQS, 0:1])
            t_pqs = mp.tile([NUM_PREFILLS, 1], i32)
            nc.sync.dma_start(out=t_pqs[:], in_=qsl[NUM_DECODES:NUM_REQS, 0:1])
            t_pqe = mp.tile([NUM_PREFILLS, 1], i32)
            nc.sync.dma_start(out=t_pqe[:],
                              in_=qsl[NUM_DECODES + 1:NUM_REQS + 1, 0:1])
            t_pql = mp.tile([NUM_PREFILLS, 1], i32)
            nc.vector.tensor_tensor(out=t_pql[:], in0=t_pqe[:], in1=t_pqs[:],
                                    op=A.subtract)
            t_ppre = mp.tile([NUM_PREFILLS, 1], i32)
            nc.vector.tensor_tensor(out=t_ppre[:], in0=t_ps[:], in1=t_pql[:],
                                    op=A.subtract)
            nc.vector.tensor_scalar(out=t_ppre[:], in0=t_ppre[:],
                                    scalar1=WINDOW - 1, scalar2=None, op0=A.min)
            t_pgl = mp.tile([NUM_PREFILLS, 1], i32)
            nc.vector.tensor_tensor(out=t_pgl[:], in0=t_pql[:], in1=t_ppre[:],
                                    op=A.add)
            nc.sync.dma_start(out=o_pgl[:], in_=t_pgl[:])

    nc.compile()
    return nc


def get_nc():
    if "nc" not in _CACHE:
        _CACHE["nc"] = _build()
    return _CACHE["nc"]


def make_in_maps(kv_cache, seq_lens, query_start_loc, token_to_req_indices,
                 slot_mapping, block_table):
    kv = np.ascontiguousarray(np.asarray(kv_cache, dtype=np.float32))
    sl = np.asarray(seq_lens, dtype=np.int32)
    qs = np.asarray(query_start_loc, dtype=np.int32)
    bt = np.asarray(block_table, dtype=np.int32).reshape(NUM_REQS, MAX_BLOCKS)
    ttr = np.asarray(token_to_req_indices, dtype=np.int32)
    sm = np.asarray(slot_mapping, dtype=np.int32)

    meta2 = np.ascontiguousarray(
        np.stack([sl, qs[1:]], axis=1).astype(np.int32).reshape(NUM_REQS * 2, 1))
    # block table with jax OOB clamp baked in: btfp[r, j] = bt[r, min(j, 31)]
    btfp = np.ascontiguousarray(
        bt[:, np.minimum(np.arange(BTW), MAX_BLOCKS - 1)]
        .astype(np.int32).reshape(NUM_REQS * BTW, 1))

    p = np.arange(P, dtype=np.int32)
    cw = np.ascontiguousarray(
        (((p & 3) * 128)[:, None] + np.arange(COLS, dtype=np.int32)[None, :])
        .astype(np.int32))
    cu = np.ascontiguousarray(
        np.tile(np.arange(BLOCK_SIZE, dtype=np.int32), (P, 1)))

    in_maps = []
    for c in range(N_CORES):
        toks4 = c * TPC + (p >> 2)
        m = {
            "kv": kv, "meta2": meta2, "btfp": btfp, "cw": cw, "cu": cu,
            "seqlens": np.ascontiguousarray(sl.reshape(NUM_REQS, 1)),
            "qsl": np.ascontiguousarray(qs.reshape(NUM_REQS + 1, 1)),
            "req4": np.ascontiguousarray(ttr[toks4].reshape(P, 1)),
            "sm4": np.ascontiguousarray(sm[toks4].reshape(P, 1)),
            "ctok": np.ascontiguousarray(toks4.astype(np.int32).reshape(P, 1)),
        }
        for b, (tok0, nb) in enumerate(BATCHES):
            il = p // NRUN            # pair -> local token
            k = p % NRUN              # pair -> run index
            used = p < nb * NRUN
            gtok = np.where(used, c * TPC + tok0 + il, c * TPC)  # pad -> tok 0
            dest = np.where(used, GUT + il * STRIDE + k * BLOCK_SIZE, BIG)
            m[f"preq{b}"] = np.ascontiguousarray(ttr[gtok].reshape(P, 1))
            m[f"ptok{b}"] = np.ascontiguousarray(
                gtok.astype(np.int32).reshape(P, 1))
            m[f"ppk{b}"] = np.ascontiguousarray(
                np.where(used, k, 0).astype(np.int32).reshape(P, 1))
            m[f"ppdest{b}"] = np.ascontiguousarray(
                dest.astype(np.int32).reshape(P, 1))
            m[f"ppsm{b}"] = np.ascontiguousarray(sm[gtok].reshape(P, 1))
        in_maps.append(m)
    return in_maps


def assemble(results):
    swa_indices = np.concatenate(
        [results[c]["o_sidx"].reshape(TPC, WINDOW) for c in range(N_CORES)],
        axis=0)
    swa_lens = np.concatenate(
        [results[c]["o_slen"].reshape(TPC) for c in range(N_CORES)], axis=0)
    pgl = results[0]["o_pgl"].reshape(NUM_PREFILLS)
    gathered = np.empty((NUM_DECODES, WINDOW, HEAD), dtype=np.float32)
    for c in range(N_CORES):
        for b, (tok0, nb) in enumerate(BATCHES):
            arr = results[c][f"o_g{b}"]
            for il in range(nb):
                r0 = GUT + il * STRIDE
                gathered[c * TPC + tok0 + il] = arr[r0:r0 + WINDOW]
    return (swa_indices.astype(np.int32), swa_lens.astype(np.int32),
            pgl.astype(np.int32), gathered)


def kernel(kv_cache, seq_lens, query_start_loc, token_to_req_indices,
           slot_mapping, block_table, window_size=WINDOW, block_size=BLOCK_SIZE,
           num_decodes=NUM_DECODES, num_prefills=NUM_PREFILLS,
           num_decode_tokens=NUM_DECODES, **_ignored):
    assert int(window_size) == WINDOW and int(block_size) == BLOCK_SIZE
    assert int(num_decodes) == NUM_DECODES and int(num_prefills) == NUM_PREFILLS
    nc = get_nc()
    in_maps = make_in_maps(kv_cache, seq_lens, query_start_loc,
                           token_to_req_indices, slot_mapping, block_table)
    res = run_bass_kernel_spmd(nc, in_maps, list(range(N_CORES)))
    _CACHE["last_result"] = res
    return assemble(res.results)


# revision 10
# speedup vs baseline: 1.1889x; 1.1889x over previous
"""Trainium2 Bass kernel for DeepseekV4 SWA cache gather (sparse_attention).

Contract: kernel(**inputs) takes FULL unsharded inputs, distributes across 8
NeuronCores (data-parallel over the 256 decode tokens, 32 per core; kv_cache /
seq_lens / query_start_loc / block_table replicated), runs a Bass/Tile kernel
via run_bass_kernel_spmd, and reassembles the full outputs:

    (swa_indices [256,512] i32, swa_lens [256] i32,
     prefill_gather_lens [8] i32, gathered [256,512,256] f32)

Gather strategy (general, paged): per token the 512-slot window decomposes
into <=9 runs, each contiguous inside one KV block. HW indirect DMA moves,
per partition, ONE dynamic offset + a contiguous extent. So per (token, k):
  read:    64 rows from kv[btfp[req, A+k]*64]        (block aligned; btfp is
           the block table with jax's OOB clamp pre-baked as column padding)
  mask:    rows with window pos >= swa_len zeroed in SBUF
  scatter: 64 rows to output row 64 + i*576 + 64k - r  (gutter-padded layout,
           spills from window edges land in 64-row gutters between tokens)
Host extracts the 512-row token slices. swa_indices are computed on-device
with a one-hot-over-9 select in a [128,128] layout.

Self-contained: all shapes/constants hardcoded.
"""

import numpy as np

import concourse.bass as bass
import concourse.mybir as mybir
import concourse.tile as tile
import concourse.bacc as bacc
from concourse.bass import IndirectOffsetOnAxis
from concourse.bass_utils import run_bass_kernel_spmd

# ---- problem constants (hardcoded per contract) ----
NUM_DECODES = 256
NUM_PREFILLS = 8
WINDOW = 512
BLOCK_SIZE = 64
MAX_BLOCKS = 32
HEAD = 256
NUM_REQS = NUM_DECODES + NUM_PREFILLS          # 264
NUM_SLOTS = NUM_REQS * MAX_BLOCKS * BLOCK_SIZE  # 540672

N_CORES = 8
TPC = NUM_DECODES // N_CORES                   # 32 tokens per core
P = 128
COLS = TPC * WINDOW // P                       # 128 (swa_indices layout)
NRUN = 9                                       # blocks per token window
BTW = MAX_BLOCKS + 33                          # 65: clamp-padded bt row width
STRIDE = NRUN * BLOCK_SIZE                     # 576-row per-token pitch
GUT = BLOCK_SIZE                               # 64-row head gutter
BATCHES = [(0, 12), (12, 12), (24, 8)]         # (start token, n tokens)
BIG = 1 << 20

i32 = mybir.dt.int32
f32 = mybir.dt.float32

_CACHE = {}


def _og_rows(nb):
    return GUT + nb * STRIDE


def _build():
    nc = bacc.Bacc("TRN2", target_bir_lowering=False, debug=False,
                   num_devices=N_CORES)
    A = mybir.AluOpType

    # replicated inputs
    kv = nc.dram_tensor("kv", [NUM_SLOTS, HEAD], f32, kind="ExternalInput")
    # meta2[r] = (seq_lens[r], query_start_loc[r+1]) zipped on host
    meta2 = nc.dram_tensor("meta2", [NUM_REQS * 2, 1], i32, kind="ExternalInput")
    qsl = nc.dram_tensor("qsl", [NUM_REQS + 1, 1], i32, kind="ExternalInput")
    seqlens = nc.dram_tensor("seqlens", [NUM_REQS, 1], i32, kind="ExternalInput")
    btfp = nc.dram_tensor("btfp", [NUM_REQS * BTW, 1], i32, kind="ExternalInput")
    cw = nc.dram_tensor("cw", [P, COLS], i32, kind="ExternalInput")
    cu = nc.dram_tensor("cu", [P, BLOCK_SIZE], i32, kind="ExternalInput")
    # per-core (token x4 layout for swa_indices path)
    req4 = nc.dram_tensor("req4", [P, 1], i32, kind="ExternalInput")
    sm4 = nc.dram_tensor("sm4", [P, 1], i32, kind="ExternalInput")
    ctok = nc.dram_tensor("ctok", [P, 1], i32, kind="ExternalInput")
    # per-core pair layouts, one per batch: p <-> (il, k) = (p//9, p%9)
    preq, ptok, ppk, ppdest, ppsm = [], [], [], [], []
    for b, _ in enumerate(BATCHES):
        preq.append(nc.dram_tensor(f"preq{b}", [P, 1], i32, kind="ExternalInput"))
        ptok.append(nc.dram_tensor(f"ptok{b}", [P, 1], i32, kind="ExternalInput"))
        ppk.append(nc.dram_tensor(f"ppk{b}", [P, 1], i32, kind="ExternalInput"))
        ppdest.append(nc.dram_tensor(f"ppdest{b}", [P, 1], i32,
                                     kind="ExternalInput"))
        ppsm.append(nc.dram_tensor(f"ppsm{b}", [P, 1], i32, kind="ExternalInput"))

    # outputs
    o_sidx = nc.dram_tensor("o_sidx", [P, COLS], i32, kind="ExternalOutput")
    o_slen = nc.dram_tensor("o_slen", [TPC, 1], i32, kind="ExternalOutput")
    o_pgl = nc.dram_tensor("o_pgl", [NUM_PREFILLS, 1], i32, kind="ExternalOutput")
    o_g = [nc.dram_tensor(f"o_g{b}", [_og_rows(nb), HEAD], f32,
                          kind="ExternalOutput")
           for b, (_, nb) in enumerate(BATCHES)]

    with tile.TileContext(nc) as tc:
        with tc.tile_pool(name="meta", bufs=1) as mp, \
             tc.tile_pool(name="gp", bufs=2) as gp:
            # ========== phase 0: loads ==========
            t_req4 = mp.tile([P, 1], i32)
            nc.sync.dma_start(out=t_req4[:], in_=req4[:])
            t_sm4 = mp.tile([P, 1], i32)
            nc.sync.dma_start(out=t_sm4[:], in_=sm4[:])
            t_ctok = mp.tile([P, 1], i32)
            nc.sync.dma_start(out=t_ctok[:], in_=ctok[:])
            t_cw = mp.tile([P, COLS], i32)
            nc.sync.dma_start(out=t_cw[:], in_=cw[:])
            t_cu = mp.tile([P, BLOCK_SIZE], i32)
            nc.sync.dma_start(out=t_cu[:], in_=cu[:])
            bt_req, bt_tok, bt_k, bt_dst0, bt_sm = [], [], [], [], []
            for b, _ in enumerate(BATCHES):
                x = mp.tile([P, 1], i32, tag=f"breq{b}")
                nc.sync.dma_start(out=x[:], in_=preq[b][:])
                bt_req.append(x)
                x = mp.tile([P, 1], i32, tag=f"btok{b}")
                nc.sync.dma_start(out=x[:], in_=ptok[b][:])
                bt_tok.append(x)
                x = mp.tile([P, 1], i32, tag=f"bk{b}")
                nc.sync.dma_start(out=x[:], in_=ppk[b][:])
                bt_k.append(x)
                x = mp.tile([P, 1], i32, tag=f"bdst0{b}")
                nc.sync.dma_start(out=x[:], in_=ppdest[b][:])
                bt_dst0.append(x)
                x = mp.tile([P, 1], i32, tag=f"bsm{b}")
                nc.sync.dma_start(out=x[:], in_=ppsm[b][:])
                bt_sm.append(x)

            # ========== phase 1: batch metadata (pair layout) ==========
            b_ridx, b_dst, b_mask = [], [], []
            for b, (tok0, nb) in enumerate(BATCHES):
                b_r2 = mp.tile([P, 1], i32, tag=f"br2{b}")
                nc.vector.tensor_scalar(out=b_r2[:], in0=bt_req[b][:],
                                        scalar1=2, scalar2=None, op0=A.mult)
                b_m2 = mp.tile([P, 2], i32, tag=f"bm2{b}")
                nc.gpsimd.indirect_dma_start(
                    out=b_m2[:], out_offset=None, in_=meta2[:],
                    in_offset=IndirectOffsetOnAxis(ap=b_r2[:, :1], axis=0))
                b_pos = mp.tile([P, 1], i32, tag=f"bpos{b}")
                # pos = seq - qe + tok   (qs cancels from seq - qlen - qs)
                nc.vector.tensor_tensor(out=b_pos[:], in0=b_m2[:, 0:1],
                                        in1=b_m2[:, 1:2], op=A.subtract)
                nc.vector.tensor_tensor(out=b_pos[:], in0=b_pos[:],
                                        in1=bt_tok[b][:], op=A.add)
                b_a = mp.tile([P, 1], i32, tag=f"ba{b}")
                nc.vector.tensor_scalar(out=b_a[:], in0=b_pos[:],
                                        scalar1=WINDOW - 1, scalar2=0,
                                        op0=A.subtract, op1=A.max)
                b_len = mp.tile([P, 1], i32, tag=f"blen{b}")
                nc.vector.tensor_scalar(out=b_len[:], in0=b_pos[:],
                                        scalar1=1, scalar2=WINDOW,
                                        op0=A.add, op1=A.min)
                b_vm = mp.tile([P, 1], i32, tag=f"bvm{b}")
                nc.vector.tensor_scalar(out=b_vm[:], in0=bt_sm[b][:],
                                        scalar1=0, scalar2=None, op0=A.is_ge)
                nc.vector.tensor_tensor(out=b_len[:], in0=b_len[:],
                                        in1=b_vm[:], op=A.mult)
                b_A = mp.tile([P, 1], i32, tag=f"bA{b}")
                nc.vector.tensor_scalar(out=b_A[:], in0=b_a[:],
                                        scalar1=6, scalar2=None,
                                        op0=A.arith_shift_right)
                b_r = mp.tile([P, 1], i32, tag=f"br{b}")
                nc.vector.tensor_scalar(out=b_r[:], in0=b_a[:],
                                        scalar1=BLOCK_SIZE - 1, scalar2=None,
                                        op0=A.bitwise_and)
                # btfp flat index = req*65 + A + k (clamp pre-baked in btfp)
                b_ki = mp.tile([P, 1], i32, tag=f"bki{b}")
                nc.vector.tensor_tensor(out=b_ki[:], in0=b_A[:],
                                        in1=bt_k[b][:], op=A.add)
                b_r65 = mp.tile([P, 1], i32, tag=f"br65{b}")
                nc.vector.tensor_scalar(out=b_r65[:], in0=bt_req[b][:],
                                        scalar1=BTW, scalar2=None, op0=A.mult)
                nc.vector.tensor_tensor(out=b_ki[:], in0=b_ki[:], in1=b_r65[:],
                                        op=A.add)
                b_blk = mp.tile([P, 1], i32, tag=f"bblk{b}")
                nc.gpsimd.indirect_dma_start(
                    out=b_blk[:], out_offset=None, in_=btfp[:],
                    in_offset=IndirectOffsetOnAxis(ap=b_ki[:, :1], axis=0))
                x = mp.tile([P, 1], i32, tag=f"bridx{b}")
                nc.vector.tensor_scalar(out=x[:], in0=b_blk[:],
                                        scalar1=BLOCK_SIZE, scalar2=None,
                                        op0=A.mult)
                b_ridx.append(x)
                # dest row = (64 + il*576 + 64k) - r   (pad lanes: BIG)
                x = mp.tile([P, 1], i32, tag=f"bdst{b}")
                nc.vector.tensor_tensor(out=x[:], in0=bt_dst0[b][:],
                                        in1=b_r[:], op=A.subtract)
                b_dst.append(x)
                # keep-threshold: row u kept iff u < thr = len + r - 64k
                b_thr = mp.tile([P, 1], i32, tag=f"bthr{b}")
                nc.vector.tensor_tensor(out=b_thr[:], in0=b_len[:], in1=b_r[:],
                                        op=A.add)
                b_k64 = mp.tile([P, 1], i32, tag=f"bk64{b}")
                nc.vector.tensor_scalar(out=b_k64[:], in0=bt_k[b][:],
                                        scalar1=BLOCK_SIZE, scalar2=None,
                                        op0=A.mult)
                nc.vector.tensor_tensor(out=b_thr[:], in0=b_thr[:],
                                        in1=b_k64[:], op=A.subtract)
                b_thrf = mp.tile([P, 1], f32, tag=f"bthrf{b}")
                nc.vector.tensor_copy(out=b_thrf[:], in_=b_thr[:])
                x = mp.tile([P, BLOCK_SIZE], i32, tag=f"bmask{b}")
                nc.vector.tensor_scalar(out=x[:], in0=t_cu[:],
                                        scalar1=b_thrf[:, :1], scalar2=None,
                                        op0=A.is_lt)
                b_mask.append(x)
                # pad lanes: skip the 64KB read entirely (idx -> OOB)
                b_pad = mp.tile([P, 1], i32, tag=f"bpad{b}")
                nc.vector.tensor_scalar(out=b_pad[:], in0=bt_dst0[b][:],
                                        scalar1=BIG - 1, scalar2=BIG,
                                        op0=A.is_gt, op1=A.mult)
                nc.vector.tensor_tensor(out=b_ridx[b][:], in0=b_ridx[b][:],
                                        in1=b_pad[:], op=A.add)

            # ========== phase 2/3/4: read -> mask -> scatter ==========
            gts = []
            for b, (tok0, nb) in enumerate(BATCHES):
                gt = gp.tile([P, BLOCK_SIZE * HEAD], f32, tag="gblk")
                nc.gpsimd.indirect_dma_start(
                    out=gt[:], out_offset=None, in_=kv[:],
                    in_offset=IndirectOffsetOnAxis(ap=b_ridx[b][:, :1], axis=0),
                    bounds_check=NUM_SLOTS - 1, oob_is_err=False)
                gts.append(gt)
            for b, (tok0, nb) in enumerate(BATCHES):
                g3 = gts[b][:].rearrange("p (n d) -> p n d", n=BLOCK_SIZE)
                # zero rows u >= thr
                nc.vector.tensor_tensor(
                    out=g3, in0=g3,
                    in1=b_mask[b][:].to_broadcast([P, BLOCK_SIZE, HEAD]),
                    op=A.mult)
            for b, (tok0, nb) in enumerate(BATCHES):
                nc.gpsimd.indirect_dma_start(
                    out=o_g[b][:], out_offset=IndirectOffsetOnAxis(
                        ap=b_dst[b][:, :1], axis=0),
                    in_=gts[b][:], in_offset=None,
                    bounds_check=_og_rows(nb) - 1, oob_is_err=False)

            # ========== swa_indices path (token x4 layout) ==========
            t_r2 = mp.tile([P, 1], i32)
            nc.vector.tensor_scalar(out=t_r2[:], in0=t_req4[:],
                                    scalar1=2, scalar2=None, op0=A.mult)
            t_m2 = mp.tile([P, 2], i32)
            nc.gpsimd.indirect_dma_start(
                out=t_m2[:], out_offset=None, in_=meta2[:],
                in_offset=IndirectOffsetOnAxis(ap=t_r2[:, :1], axis=0))
            t_pos = mp.tile([P, 1], i32)
            nc.vector.tensor_tensor(out=t_pos[:], in0=t_m2[:, 0:1],
                                    in1=t_m2[:, 1:2], op=A.subtract)
            nc.vector.tensor_tensor(out=t_pos[:], in0=t_pos[:], in1=t_ctok[:],
                                    op=A.add)
            t_a = mp.tile([P, 1], i32)
            nc.vector.tensor_scalar(out=t_a[:], in0=t_pos[:],
                                    scalar1=WINDOW - 1, scalar2=0,
                                    op0=A.subtract, op1=A.max)
            t_len = mp.tile([P, 1], i32)
            nc.vector.tensor_scalar(out=t_len[:], in0=t_pos[:],
                                    scalar1=1, scalar2=WINDOW,
                                    op0=A.add, op1=A.min)
            t_vm = mp.tile([P, 1], i32)
            nc.vector.tensor_scalar(out=t_vm[:], in0=t_sm4[:],
                                    scalar1=0, scalar2=None, op0=A.is_ge)
            nc.vector.tensor_tensor(out=t_len[:], in0=t_len[:], in1=t_vm[:],
                                    op=A.mult)
            t_lenf = mp.tile([P, 1], f32)
            nc.vector.tensor_copy(out=t_lenf[:], in_=t_len[:])
            t_A = mp.tile([P, 1], i32)
            nc.vector.tensor_scalar(out=t_A[:], in0=t_a[:],
                                    scalar1=6, scalar2=None,
                                    op0=A.arith_shift_right)
            t_r = mp.tile([P, 1], i32)
            nc.vector.tensor_scalar(out=t_r[:], in0=t_a[:],
                                    scalar1=BLOCK_SIZE - 1, scalar2=None,
                                    op0=A.bitwise_and)
            # bt9 = btfp[req*65 + A .. +9] in one consecutive-extent gather
            t_i9 = mp.tile([P, 1], i32)
            nc.vector.tensor_scalar(out=t_i9[:], in0=t_req4[:],
                                    scalar1=BTW, scalar2=None, op0=A.mult)
            nc.vector.tensor_tensor(out=t_i9[:], in0=t_i9[:], in1=t_A[:],
                                    op=A.add)
            t_bt9 = mp.tile([P, NRUN], i32)
            nc.gpsimd.indirect_dma_start(
                out=t_bt9[:], out_offset=None, in_=btfp[:],
                in_offset=IndirectOffsetOnAxis(ap=t_i9[:, :1], axis=0))
            t_bt9f = mp.tile([P, NRUN], f32)
            nc.vector.tensor_scalar(out=t_bt9f[:], in0=t_bt9[:],
                                    scalar1=float(BLOCK_SIZE), scalar2=None,
                                    op0=A.mult)
            # km=(r+w)>>6, lo=(a+w)&63, one-hot over k
            t_rw = mp.tile([P, COLS], i32)
            nc.vector.tensor_tensor(out=t_rw[:], in0=t_cw[:],
                                    in1=t_r[:, :1].to_broadcast([P, COLS]),
                                    op=A.add)
            t_km = mp.tile([P, COLS], i32)
            nc.vector.tensor_scalar(out=t_km[:], in0=t_rw[:],
                                    scalar1=6, scalar2=None,
                                    op0=A.arith_shift_right)
            t_aw = mp.tile([P, COLS], i32)
            nc.vector.tensor_tensor(out=t_aw[:], in0=t_cw[:],
                                    in1=t_a[:, :1].to_broadcast([P, COLS]),
                                    op=A.add)
            t_lo = mp.tile([P, COLS], i32)
            nc.vector.tensor_scalar(out=t_lo[:], in0=t_aw[:],
                                    scalar1=BLOCK_SIZE - 1, scalar2=None,
                                    op0=A.bitwise_and)
            t_acc = mp.tile([P, COLS], i32)
            nc.vector.tensor_scalar(out=t_acc[:], in0=t_km[:],
                                    scalar1=0, scalar2=t_bt9f[:, 0:1],
                                    op0=A.is_equal, op1=A.mult)
            t_term = mp.tile([P, COLS], i32)
            for k in range(1, NRUN):
                nc.vector.tensor_scalar(out=t_term[:], in0=t_km[:],
                                        scalar1=k, scalar2=t_bt9f[:, k:k + 1],
                                        op0=A.is_equal, op1=A.mult)
                nc.vector.tensor_tensor(out=t_acc[:], in0=t_acc[:],
                                        in1=t_term[:], op=A.add)
            t_slot = mp.tile([P, COLS], i32)
            nc.vector.tensor_tensor(out=t_slot[:], in0=t_acc[:], in1=t_lo[:],
                                    op=A.add)
            t_inw = mp.tile([P, COLS], i32)
            nc.vector.tensor_scalar(out=t_inw[:], in0=t_cw[:],
                                    scalar1=t_lenf[:, :1], scalar2=None,
                                    op0=A.is_lt)
            t_sidx = mp.tile([P, COLS], i32)
            nc.vector.tensor_scalar(out=t_sidx[:], in0=t_slot[:],
                                    scalar1=1, scalar2=None, op0=A.add)
            nc.vector.tensor_tensor(out=t_sidx[:], in0=t_sidx[:], in1=t_inw[:],
                                    op=A.mult)
            nc.vector.tensor_scalar(out=t_sidx[:], in0=t_sidx[:],
                                    scalar1=1, scalar2=None, op0=A.subtract)
            nc.sync.dma_start(out=o_sidx[:], in_=t_sidx[:])

            # swa_lens out: [128,1] -> [32,4] -> column 0
            t_l32 = mp.tile([TPC, 4], i32)
            nc.sync.dma_start(out=t_l32[:], in_=t_len[:])
            nc.sync.dma_start(out=o_slen[:], in_=t_l32[:, 0:1])

            # prefill_gather_lens (identical on all cores; host uses core 0)
            t_ps = mp.tile([NUM_PREFILLS, 1], i32)
            nc.sync.dma_start(out=t_ps[:], in_=seqlens[NUM_DECODES:NUM_REQS, 0:1])
            t_pqs = mp.tile([NUM_PREFILLS, 1], i32)
            nc.sync.dma_start(out=t_pqs[:], in_=qsl[NUM_DECODES:NUM_REQS, 0:1])
            t_pqe = mp.tile([NUM_PREFILLS, 1], i32)
            nc.sync.dma_start(out=t_pqe[:],
                              in_=qsl[NUM_DECODES + 1:NUM_REQS + 1, 0:1])
            t_pql = mp.tile([NUM_PREFILLS, 1], i32)
            nc.vector.tensor_tensor(out=t_pql[:], in0=t_pqe[:], in1=t_pqs[:],
                                    op=A.subtract)
            t_ppre = mp.tile([NUM_PREFILLS, 1], i32)
            nc.vector.tensor_tensor(out=t_ppre[:], in0=t_ps[:], in1=t_pql[:],
                                    op=A.subtract)
            nc.vector.tensor_scalar(out=t_ppre[:], in0=t_ppre[:],
                                    scalar1=WINDOW - 1, scalar2=None, op0=A.min)
            t_pgl = mp.tile([NUM_PREFILLS, 1], i32)
            nc.vector.tensor_tensor(out=t_pgl[:], in0=t_pql[:], in1=t_ppre[:],
                                    op=A.add)
            nc.sync.dma_start(out=o_pgl[:], in_=t_pgl[:])

    nc.compile()
    return nc


def get_nc():
    if "nc" not in _CACHE:
        _CACHE["nc"] = _build()
    return _CACHE["nc"]


def make_in_maps(kv_cache, seq_lens, query_start_loc, token_to_req_indices,
                 slot_mapping, block_table):
    kv = np.ascontiguousarray(np.asarray(kv_cache, dtype=np.float32))
    sl = np.asarray(seq_lens, dtype=np.int32)
    qs = np.asarray(query_start_loc, dtype=np.int32)
    bt = np.asarray(block_table, dtype=np.int32).reshape(NUM_REQS, MAX_BLOCKS)
    ttr = np.asarray(token_to_req_indices, dtype=np.int32)
    sm = np.asarray(slot_mapping, dtype=np.int32)

    meta2 = np.ascontiguousarray(
        np.stack([sl, qs[1:]], axis=1).astype(np.int32).reshape(NUM_REQS * 2, 1))
    # block table with jax OOB clamp baked in: btfp[r, j] = bt[r, min(j, 31)]
    btfp = np.ascontiguousarray(
        bt[:, np.minimum(np.arange(BTW), MAX_BLOCKS - 1)]
        .astype(np.int32).reshape(NUM_REQS * BTW, 1))

    p = np.arange(P, dtype=np.int32)
    cw = np.ascontiguousarray(
        (((p & 3) * 128)[:, None] + np.arange(COLS, dtype=np.int32)[None, :])
        .astype(np.int32))
    cu = np.ascontiguousarray(
        np.tile(np.arange(BLOCK_SIZE, dtype=np.int32), (P, 1)))

    in_maps = []
    for c in range(N_CORES):
        toks4 = c * TPC + (p >> 2)
        m = {
            "kv": kv, "meta2": meta2, "btfp": btfp, "cw": cw, "cu": cu,
            "seqlens": np.ascontiguousarray(sl.reshape(NUM_REQS, 1)),
            "qsl": np.ascontiguousarray(qs.reshape(NUM_REQS + 1, 1)),
            "req4": np.ascontiguousarray(ttr[toks4].reshape(P, 1)),
            "sm4": np.ascontiguousarray(sm[toks4].reshape(P, 1)),
            "ctok": np.ascontiguousarray(toks4.astype(np.int32).reshape(P, 1)),
        }
        for b, (tok0, nb) in enumerate(BATCHES):
            il = p // NRUN            # pair -> local token
            k = p % NRUN              # pair -> run index
            used = p < nb * NRUN
            gtok = np.where(used, c * TPC + tok0 + il, c * TPC)  # pad -> tok 0
            dest = np.where(used, GUT + il * STRIDE + k * BLOCK_SIZE, BIG)
            m[f"preq{b}"] = np.ascontiguousarray(ttr[gtok].reshape(P, 1))
            m[f"ptok{b}"] = np.ascontiguousarray(
                gtok.astype(np.int32).reshape(P, 1))
            m[f"ppk{b}"] = np.ascontiguousarray(
                np.where(used, k, 0).astype(np.int32).reshape(P, 1))
            m[f"ppdest{b}"] = np.ascontiguousarray(
                dest.astype(np.int32).reshape(P, 1))
            m[f"ppsm{b}"] = np.ascontiguousarray(sm[gtok].reshape(P, 1))
        in_maps.append(m)
    return in_maps


def assemble(results):
    swa_indices = np.concatenate(
        [results[c]["o_sidx"].reshape(TPC, WINDOW) for c in range(N_CORES)],
        axis=0)
    swa_lens = np.concatenate(
        [results[c]["o_slen"].reshape(TPC) for c in range(N_CORES)], axis=0)
    pgl = results[0]["o_pgl"].reshape(NUM_PREFILLS)
    gathered = np.empty((NUM_DECODES, WINDOW, HEAD), dtype=np.float32)
    for c in range(N_CORES):
        for b, (tok0, nb) in enumerate(BATCHES):
            arr = results[c][f"o_g{b}"]
            for il in range(nb):
                r0 = GUT + il * STRIDE
                gathered[c * TPC + tok0 + il] = arr[r0:r0 + WINDOW]
    return (swa_indices.astype(np.int32), swa_lens.astype(np.int32),
            pgl.astype(np.int32), gathered)


def kernel(kv_cache, seq_lens, query_start_loc, token_to_req_indices,
           slot_mapping, block_table, window_size=WINDOW, block_size=BLOCK_SIZE,
           num_decodes=NUM_DECODES, num_prefills=NUM_PREFILLS,
           num_decode_tokens=NUM_DECODES, **_ignored):
    assert int(window_size) == WINDOW and int(block_size) == BLOCK_SIZE
    assert int(num_decodes) == NUM_DECODES and int(num_prefills) == NUM_PREFILLS
    nc = get_nc()
    in_maps = make_in_maps(kv_cache, seq_lens, query_start_loc,
                           token_to_req_indices, slot_mapping, block_table)
    res = run_bass_kernel_spmd(nc, in_maps, list(range(N_CORES)))
    _CACHE["last_result"] = res
    return assemble(res.results)


# revision 11
# speedup vs baseline: 1.1915x; 1.0022x over previous
"""Trainium2 Bass kernel for DeepseekV4 SWA cache gather (sparse_attention).

Contract: kernel(**inputs) takes FULL unsharded inputs, distributes across 8
NeuronCores (data-parallel over the 256 decode tokens, 32 per core; kv_cache /
seq_lens / query_start_loc / block_table replicated), runs a Bass/Tile kernel
via run_bass_kernel_spmd, and reassembles the full outputs:

    (swa_indices [256,512] i32, swa_lens [256] i32,
     prefill_gather_lens [8] i32, gathered [256,512,256] f32)

Gather strategy (general, paged): per token the 512-slot window decomposes
into <=9 runs, each contiguous inside one KV block. HW indirect DMA moves,
per partition, ONE dynamic offset + a contiguous extent. So per (token, k):
  read:    64 rows from kv[btfp[req, A+k]*64]        (block aligned; btfp is
           the block table with jax's OOB clamp pre-baked as column padding)
  mask:    rows with window pos >= swa_len zeroed in SBUF
  scatter: 64 rows to output row 64 + i*576 + 64k - r  (gutter-padded layout,
           spills from window edges land in 64-row gutters between tokens)
Host extracts the 512-row token slices. swa_indices are computed on-device
with a one-hot-over-9 select in a [128,128] layout.

Self-contained: all shapes/constants hardcoded.
"""

import numpy as np

import concourse.bass as bass
import concourse.mybir as mybir
import concourse.tile as tile
import concourse.bacc as bacc
from concourse.bass import IndirectOffsetOnAxis
from concourse.bass_utils import run_bass_kernel_spmd

# ---- problem constants (hardcoded per contract) ----
NUM_DECODES = 256
NUM_PREFILLS = 8
WINDOW = 512
BLOCK_SIZE = 64
MAX_BLOCKS = 32
HEAD = 256
NUM_REQS = NUM_DECODES + NUM_PREFILLS          # 264
NUM_SLOTS = NUM_REQS * MAX_BLOCKS * BLOCK_SIZE  # 540672

N_CORES = 8
TPC = NUM_DECODES // N_CORES                   # 32 tokens per core
P = 128
COLS = TPC * WINDOW // P                       # 128 (swa_indices layout)
NRUN = 9                                       # blocks per token window
BTW = MAX_BLOCKS + 33                          # 65: clamp-padded bt row width
STRIDE = NRUN * BLOCK_SIZE                     # 576-row per-token pitch
GUT = BLOCK_SIZE                               # 64-row head gutter
BATCHES = [(0, 12), (12, 12), (24, 8)]         # (start token, n tokens)
BIG = 1 << 20

i32 = mybir.dt.int32
f32 = mybir.dt.float32

_CACHE = {}


def _og_rows(nb):
    return GUT + nb * STRIDE


def _build():
    nc = bacc.Bacc("TRN2", target_bir_lowering=False, debug=False,
                   num_devices=N_CORES)
    A = mybir.AluOpType

    # replicated inputs
    kv = nc.dram_tensor("kv", [NUM_SLOTS, HEAD], f32, kind="ExternalInput")
    # meta2[r] = (seq_lens[r], query_start_loc[r+1]) zipped on host
    meta2 = nc.dram_tensor("meta2", [NUM_REQS * 2, 1], i32, kind="ExternalInput")
    qsl = nc.dram_tensor("qsl", [NUM_REQS + 1, 1], i32, kind="ExternalInput")
    seqlens = nc.dram_tensor("seqlens", [NUM_REQS, 1], i32, kind="ExternalInput")
    btfp = nc.dram_tensor("btfp", [NUM_REQS * BTW, 1], i32, kind="ExternalInput")
    cw = nc.dram_tensor("cw", [P, COLS], i32, kind="ExternalInput")
    cu = nc.dram_tensor("cu", [P, BLOCK_SIZE], i32, kind="ExternalInput")
    # per-core (token x4 layout for swa_indices path)
    req4 = nc.dram_tensor("req4", [P, 1], i32, kind="ExternalInput")
    sm4 = nc.dram_tensor("sm4", [P, 1], i32, kind="ExternalInput")
    ctok = nc.dram_tensor("ctok", [P, 1], i32, kind="ExternalInput")
    # per-core pair layouts, one per batch: p <-> (il, k) = (p//9, p%9)
    preq, ptok, ppk, ppdest, ppsm = [], [], [], [], []
    for b, _ in enumerate(BATCHES):
        preq.append(nc.dram_tensor(f"preq{b}", [P, 1], i32, kind="ExternalInput"))
        ptok.append(nc.dram_tensor(f"ptok{b}", [P, 1], i32, kind="ExternalInput"))
        ppk.append(nc.dram_tensor(f"ppk{b}", [P, 1], i32, kind="ExternalInput"))
        ppdest.append(nc.dram_tensor(f"ppdest{b}", [P, 1], i32,
                                     kind="ExternalInput"))
        ppsm.append(nc.dram_tensor(f"ppsm{b}", [P, 1], i32, kind="ExternalInput"))

    # outputs
    o_sidx = nc.dram_tensor("o_sidx", [P, COLS], i32, kind="ExternalOutput")
    o_slen = nc.dram_tensor("o_slen", [TPC, 1], i32, kind="ExternalOutput")
    o_pgl = nc.dram_tensor("o_pgl", [NUM_PREFILLS, 1], i32, kind="ExternalOutput")
    o_g = [nc.dram_tensor(f"o_g{b}", [_og_rows(nb), HEAD], f32,
                          kind="ExternalOutput")
           for b, (_, nb) in enumerate(BATCHES)]

    with tile.TileContext(nc) as tc:
        with tc.tile_pool(name="meta", bufs=1) as mp, \
             tc.tile_pool(name="gp", bufs=2) as gp:
            # ========== phase 0: loads ==========
            t_req4 = mp.tile([P, 1], i32)
            nc.sync.dma_start(out=t_req4[:], in_=req4[:])
            t_sm4 = mp.tile([P, 1], i32)
            nc.sync.dma_start(out=t_sm4[:], in_=sm4[:])
            t_ctok = mp.tile([P, 1], i32)
            nc.sync.dma_start(out=t_ctok[:], in_=ctok[:])
            t_cw = mp.tile([P, COLS], i32)
            nc.sync.dma_start(out=t_cw[:], in_=cw[:])
            t_cu = mp.tile([P, BLOCK_SIZE], i32)
            nc.sync.dma_start(out=t_cu[:], in_=cu[:])
            bt_req, bt_tok, bt_k, bt_dst0, bt_sm = [], [], [], [], []
            for b, _ in enumerate(BATCHES):
                x = mp.tile([P, 1], i32, tag=f"breq{b}")
                nc.sync.dma_start(out=x[:], in_=preq[b][:])
                bt_req.append(x)
                x = mp.tile([P, 1], i32, tag=f"btok{b}")
                nc.sync.dma_start(out=x[:], in_=ptok[b][:])
                bt_tok.append(x)
                x = mp.tile([P, 1], i32, tag=f"bk{b}")
                nc.sync.dma_start(out=x[:], in_=ppk[b][:])
                bt_k.append(x)
                x = mp.tile([P, 1], i32, tag=f"bdst0{b}")
                nc.sync.dma_start(out=x[:], in_=ppdest[b][:])
                bt_dst0.append(x)
                x = mp.tile([P, 1], i32, tag=f"bsm{b}")
                nc.sync.dma_start(out=x[:], in_=ppsm[b][:])
                bt_sm.append(x)

            # ========== phase 1: batch metadata (pair layout) ==========
            b_ridx, b_dst, b_mask = [], [], []
            for b, (tok0, nb) in enumerate(BATCHES):
                b_r2 = mp.tile([P, 1], i32, tag=f"br2{b}")
                nc.vector.tensor_scalar(out=b_r2[:], in0=bt_req[b][:],
                                        scalar1=2, scalar2=None, op0=A.mult)
                b_m2 = mp.tile([P, 2], i32, tag=f"bm2{b}")
                nc.gpsimd.indirect_dma_start(
                    out=b_m2[:], out_offset=None, in_=meta2[:],
                    in_offset=IndirectOffsetOnAxis(ap=b_r2[:, :1], axis=0))
                b_pos = mp.tile([P, 1], i32, tag=f"bpos{b}")
                # pos = seq - qe + tok   (qs cancels from seq - qlen - qs)
                nc.vector.tensor_tensor(out=b_pos[:], in0=b_m2[:, 0:1],
                                        in1=b_m2[:, 1:2], op=A.subtract)
                nc.vector.tensor_tensor(out=b_pos[:], in0=b_pos[:],
                                        in1=bt_tok[b][:], op=A.add)
                b_a = mp.tile([P, 1], i32, tag=f"ba{b}")
                nc.vector.tensor_scalar(out=b_a[:], in0=b_pos[:],
                                        scalar1=WINDOW - 1, scalar2=0,
                                        op0=A.subtract, op1=A.max)
                b_len = mp.tile([P, 1], i32, tag=f"blen{b}")
                nc.vector.tensor_scalar(out=b_len[:], in0=b_pos[:],
                                        scalar1=1, scalar2=WINDOW,
                                        op0=A.add, op1=A.min)
                b_vm = mp.tile([P, 1], i32, tag=f"bvm{b}")
                nc.vector.tensor_scalar(out=b_vm[:], in0=bt_sm[b][:],
                                        scalar1=0, scalar2=None, op0=A.is_ge)
                nc.vector.tensor_tensor(out=b_len[:], in0=b_len[:],
                                        in1=b_vm[:], op=A.mult)
                b_A = mp.tile([P, 1], i32, tag=f"bA{b}")
                nc.vector.tensor_scalar(out=b_A[:], in0=b_a[:],
                                        scalar1=6, scalar2=None,
                                        op0=A.arith_shift_right)
                b_r = mp.tile([P, 1], i32, tag=f"br{b}")
                nc.vector.tensor_scalar(out=b_r[:], in0=b_a[:],
                                        scalar1=BLOCK_SIZE - 1, scalar2=None,
                                        op0=A.bitwise_and)
                # btfp flat index = req*65 + A + k (clamp pre-baked in btfp)
                b_ki = mp.tile([P, 1], i32, tag=f"bki{b}")
                nc.vector.tensor_tensor(out=b_ki[:], in0=b_A[:],
                                        in1=bt_k[b][:], op=A.add)
                b_r65 = mp.tile([P, 1], i32, tag=f"br65{b}")
                nc.vector.tensor_scalar(out=b_r65[:], in0=bt_req[b][:],
                                        scalar1=BTW, scalar2=None, op0=A.mult)
                nc.vector.tensor_tensor(out=b_ki[:], in0=b_ki[:], in1=b_r65[:],
                                        op=A.add)
                b_blk = mp.tile([P, 1], i32, tag=f"bblk{b}")
                nc.gpsimd.indirect_dma_start(
                    out=b_blk[:], out_offset=None, in_=btfp[:],
                    in_offset=IndirectOffsetOnAxis(ap=b_ki[:, :1], axis=0))
                x = mp.tile([P, 1], i32, tag=f"bridx{b}")
                nc.vector.tensor_scalar(out=x[:], in0=b_blk[:],
                                        scalar1=BLOCK_SIZE, scalar2=None,
                                        op0=A.mult)
                b_ridx.append(x)
                # dest row = (64 + il*576 + 64k) - r   (pad lanes: BIG)
                x = mp.tile([P, 1], i32, tag=f"bdst{b}")
                nc.vector.tensor_tensor(out=x[:], in0=bt_dst0[b][:],
                                        in1=b_r[:], op=A.subtract)
                b_dst.append(x)
                # keep-threshold: row u kept iff u < thr = len + r - 64k
                b_thr = mp.tile([P, 1], i32, tag=f"bthr{b}")
                nc.vector.tensor_tensor(out=b_thr[:], in0=b_len[:], in1=b_r[:],
                                        op=A.add)
                b_k64 = mp.tile([P, 1], i32, tag=f"bk64{b}")
                nc.vector.tensor_scalar(out=b_k64[:], in0=bt_k[b][:],
                                        scalar1=BLOCK_SIZE, scalar2=None,
                                        op0=A.mult)
                nc.vector.tensor_tensor(out=b_thr[:], in0=b_thr[:],
                                        in1=b_k64[:], op=A.subtract)
                b_thrf = mp.tile([P, 1], f32, tag=f"bthrf{b}")
                nc.vector.tensor_copy(out=b_thrf[:], in_=b_thr[:])
                x = mp.tile([P, BLOCK_SIZE], i32, tag=f"bmask{b}")
                nc.vector.tensor_scalar(out=x[:], in0=t_cu[:],
                                        scalar1=b_thrf[:, :1], scalar2=None,
                                        op0=A.is_lt)
                b_mask.append(x)
                # skip pad lanes AND fully-masked blocks (thr<=0): their
                # output rows are all-zero, and External outputs are
                # pre-zeroed by the runtime -> no read, no write needed.
                b_pad = mp.tile([P, 1], i32, tag=f"bpad{b}")
                nc.vector.tensor_scalar(out=b_pad[:], in0=bt_dst0[b][:],
                                        scalar1=BIG - 1, scalar2=BIG,
                                        op0=A.is_gt, op1=A.mult)
                b_zskip = mp.tile([P, 1], i32, tag=f"bzskip{b}")
                nc.vector.tensor_scalar(out=b_zskip[:], in0=b_thr[:],
                                        scalar1=1, scalar2=BIG,
                                        op0=A.is_lt, op1=A.mult)
                nc.vector.tensor_tensor(out=b_pad[:], in0=b_pad[:],
                                        in1=b_zskip[:], op=A.add)
                nc.vector.tensor_tensor(out=b_ridx[b][:], in0=b_ridx[b][:],
                                        in1=b_pad[:], op=A.add)
                nc.vector.tensor_tensor(out=b_dst[b][:], in0=b_dst[b][:],
                                        in1=b_pad[:], op=A.add)

            # ========== phase 2/3/4: read -> mask -> scatter ==========
            gts = []
            for b, (tok0, nb) in enumerate(BATCHES):
                gt = gp.tile([P, BLOCK_SIZE * HEAD], f32, tag="gblk")
                nc.gpsimd.indirect_dma_start(
                    out=gt[:], out_offset=None, in_=kv[:],
                    in_offset=IndirectOffsetOnAxis(ap=b_ridx[b][:, :1], axis=0),
                    bounds_check=NUM_SLOTS - 1, oob_is_err=False)
                gts.append(gt)
            for b, (tok0, nb) in enumerate(BATCHES):
                g3 = gts[b][:].rearrange("p (n d) -> p n d", n=BLOCK_SIZE)
                # zero rows u >= thr
                nc.vector.tensor_tensor(
                    out=g3, in0=g3,
                    in1=b_mask[b][:].to_broadcast([P, BLOCK_SIZE, HEAD]),
                    op=A.mult)
            for b, (tok0, nb) in enumerate(BATCHES):
                nc.gpsimd.indirect_dma_start(
                    out=o_g[b][:], out_offset=IndirectOffsetOnAxis(
                        ap=b_dst[b][:, :1], axis=0),
                    in_=gts[b][:], in_offset=None,
                    bounds_check=_og_rows(nb) - 1, oob_is_err=False)

            # ========== swa_indices path (token x4 layout) ==========
            t_r2 = mp.tile([P, 1], i32)
            nc.vector.tensor_scalar(out=t_r2[:], in0=t_req4[:],
                                    scalar1=2, scalar2=None, op0=A.mult)
            t_m2 = mp.tile([P, 2], i32)
            nc.gpsimd.indirect_dma_start(
                out=t_m2[:], out_offset=None, in_=meta2[:],
                in_offset=IndirectOffsetOnAxis(ap=t_r2[:, :1], axis=0))
            t_pos = mp.tile([P, 1], i32)
            nc.vector.tensor_tensor(out=t_pos[:], in0=t_m2[:, 0:1],
                                    in1=t_m2[:, 1:2], op=A.subtract)
            nc.vector.tensor_tensor(out=t_pos[:], in0=t_pos[:], in1=t_ctok[:],
                                    op=A.add)
            t_a = mp.tile([P, 1], i32)
            nc.vector.tensor_scalar(out=t_a[:], in0=t_pos[:],
                                    scalar1=WINDOW - 1, scalar2=0,
                                    op0=A.subtract, op1=A.max)
            t_len = mp.tile([P, 1], i32)
            nc.vector.tensor_scalar(out=t_len[:], in0=t_pos[:],
                                    scalar1=1, scalar2=WINDOW,
                                    op0=A.add, op1=A.min)
            t_vm = mp.tile([P, 1], i32)
            nc.vector.tensor_scalar(out=t_vm[:], in0=t_sm4[:],
                                    scalar1=0, scalar2=None, op0=A.is_ge)
            nc.vector.tensor_tensor(out=t_len[:], in0=t_len[:], in1=t_vm[:],
                                    op=A.mult)
            t_lenf = mp.tile([P, 1], f32)
            nc.vector.tensor_copy(out=t_lenf[:], in_=t_len[:])
            t_A = mp.tile([P, 1], i32)
            nc.vector.tensor_scalar(out=t_A[:], in0=t_a[:],
                                    scalar1=6, scalar2=None,
                                    op0=A.arith_shift_right)
            t_r = mp.tile([P, 1], i32)
            nc.vector.tensor_scalar(out=t_r[:], in0=t_a[:],
                                    scalar1=BLOCK_SIZE - 1, scalar2=None,
                                    op0=A.bitwise_and)
            # bt9 = btfp[req*65 + A .. +9] in one consecutive-extent gather
            t_i9 = mp.tile([P, 1], i32)
            nc.vector.tensor_scalar(out=t_i9[:], in0=t_req4[:],
                                    scalar1=BTW, scalar2=None, op0=A.mult)
            nc.vector.tensor_tensor(out=t_i9[:], in0=t_i9[:], in1=t_A[:],
                                    op=A.add)
            t_bt9 = mp.tile([P, NRUN], i32)
            nc.gpsimd.indirect_dma_start(
                out=t_bt9[:], out_offset=None, in_=btfp[:],
                in_offset=IndirectOffsetOnAxis(ap=t_i9[:, :1], axis=0))
            t_bt9f = mp.tile([P, NRUN], f32)
            nc.vector.tensor_scalar(out=t_bt9f[:], in0=t_bt9[:],
                                    scalar1=float(BLOCK_SIZE), scalar2=None,
                                    op0=A.mult)
            # km=(r+w)>>6, lo=(a+w)&63, one-hot over k
            t_rw = mp.tile([P, COLS], i32)
            nc.vector.tensor_tensor(out=t_rw[:], in0=t_cw[:],
                                    in1=t_r[:, :1].to_broadcast([P, COLS]),
                                    op=A.add)
            t_km = mp.tile([P, COLS], i32)
            nc.vector.tensor_scalar(out=t_km[:], in0=t_rw[:],
                                    scalar1=6, scalar2=None,
                                    op0=A.arith_shift_right)
            t_aw = mp.tile([P, COLS], i32)
            nc.vector.tensor_tensor(out=t_aw[:], in0=t_cw[:],
                                    in1=t_a[:, :1].to_broadcast([P, COLS]),
                                    op=A.add)
            t_lo = mp.tile([P, COLS], i32)
            nc.vector.tensor_scalar(out=t_lo[:], in0=t_aw[:],
                                    scalar1=BLOCK_SIZE - 1, scalar2=None,
                                    op0=A.bitwise_and)
            t_acc = mp.tile([P, COLS], i32)
            nc.vector.tensor_scalar(out=t_acc[:], in0=t_km[:],
                                    scalar1=0, scalar2=t_bt9f[:, 0:1],
                                    op0=A.is_equal, op1=A.mult)
            t_term = mp.tile([P, COLS], i32)
            for k in range(1, NRUN):
                nc.vector.tensor_scalar(out=t_term[:], in0=t_km[:],
                                        scalar1=k, scalar2=t_bt9f[:, k:k + 1],
                                        op0=A.is_equal, op1=A.mult)
                nc.vector.tensor_tensor(out=t_acc[:], in0=t_acc[:],
                                        in1=t_term[:], op=A.add)
            t_slot = mp.tile([P, COLS], i32)
            nc.vector.tensor_tensor(out=t_slot[:], in0=t_acc[:], in1=t_lo[:],
                                    op=A.add)
            t_inw = mp.tile([P, COLS], i32)
            nc.vector.tensor_scalar(out=t_inw[:], in0=t_cw[:],
                                    scalar1=t_lenf[:, :1], scalar2=None,
                                    op0=A.is_lt)
            t_sidx = mp.tile([P, COLS], i32)
            nc.vector.tensor_scalar(out=t_sidx[:], in0=t_slot[:],
                                    scalar1=1, scalar2=None, op0=A.add)
            nc.vector.tensor_tensor(out=t_sidx[:], in0=t_sidx[:], in1=t_inw[:],
                                    op=A.mult)
            nc.vector.tensor_scalar(out=t_sidx[:], in0=t_sidx[:],
                                    scalar1=1, scalar2=None, op0=A.subtract)
            nc.sync.dma_start(out=o_sidx[:], in_=t_sidx[:])

            # swa_lens out: [128,1] -> [32,4] -> column 0
            t_l32 = mp.tile([TPC, 4], i32)
            nc.sync.dma_start(out=t_l32[:], in_=t_len[:])
            nc.sync.dma_start(out=o_slen[:], in_=t_l32[:, 0:1])

            # prefill_gather_lens (identical on all cores; host uses core 0)
            t_ps = mp.tile([NUM_PREFILLS, 1], i32)
            nc.sync.dma_start(out=t_ps[:], in_=seqlens[NUM_DECODES:NUM_REQS, 0:1])
            t_pqs = mp.tile([NUM_PREFILLS, 1], i32)
            nc.sync.dma_start(out=t_pqs[:], in_=qsl[NUM_DECODES:NUM_REQS, 0:1])
            t_pqe = mp.tile([NUM_PREFILLS, 1], i32)
            nc.sync.dma_start(out=t_pqe[:],
                              in_=qsl[NUM_DECODES + 1:NUM_REQS + 1, 0:1])
            t_pql = mp.tile([NUM_PREFILLS, 1], i32)
            nc.vector.tensor_tensor(out=t_pql[:], in0=t_pqe[:], in1=t_pqs[:],
                                    op=A.subtract)
            t_ppre = mp.tile([NUM_PREFILLS, 1], i32)
            nc.vector.tensor_tensor(out=t_ppre[:], in0=t_ps[:], in1=t_pql[:],
                                    op=A.subtract)
            nc.vector.tensor_scalar(out=t_ppre[:], in0=t_ppre[:],
                                    scalar1=WINDOW - 1, scalar2=None, op0=A.min)
            t_pgl = mp.tile([NUM_PREFILLS, 1], i32)
            nc.vector.tensor_tensor(out=t_pgl[:], in0=t_pql[:], in1=t_ppre[:],
                                    op=A.add)
            nc.sync.dma_start(out=o_pgl[:], in_=t_pgl[:])

    nc.compile()
    return nc


def get_nc():
    if "nc" not in _CACHE:
        _CACHE["nc"] = _build()
    return _CACHE["nc"]


def make_in_maps(kv_cache, seq_lens, query_start_loc, token_to_req_indices,
                 slot_mapping, block_table):
    kv = np.ascontiguousarray(np.asarray(kv_cache, dtype=np.float32))
    sl = np.asarray(seq_lens, dtype=np.int32)
    qs = np.asarray(query_start_loc, dtype=np.int32)
    bt = np.asarray(block_table, dtype=np.int32).reshape(NUM_REQS, MAX_BLOCKS)
    ttr = np.asarray(token_to_req_indices, dtype=np.int32)
    sm = np.asarray(slot_mapping, dtype=np.int32)

    meta2 = np.ascontiguousarray(
        np.stack([sl, qs[1:]], axis=1).astype(np.int32).reshape(NUM_REQS * 2, 1))
    # block table with jax OOB clamp baked in: btfp[r, j] = bt[r, min(j, 31)]
    btfp = np.ascontiguousarray(
        bt[:, np.minimum(np.arange(BTW), MAX_BLOCKS - 1)]
        .astype(np.int32).reshape(NUM_REQS * BTW, 1))

    p = np.arange(P, dtype=np.int32)
    cw = np.ascontiguousarray(
        (((p & 3) * 128)[:, None] + np.arange(COLS, dtype=np.int32)[None, :])
        .astype(np.int32))
    cu = np.ascontiguousarray(
        np.tile(np.arange(BLOCK_SIZE, dtype=np.int32), (P, 1)))

    in_maps = []
    for c in range(N_CORES):
        toks4 = c * TPC + (p >> 2)
        m = {
            "kv": kv, "meta2": meta2, "btfp": btfp, "cw": cw, "cu": cu,
            "seqlens": np.ascontiguousarray(sl.reshape(NUM_REQS, 1)),
            "qsl": np.ascontiguousarray(qs.reshape(NUM_REQS + 1, 1)),
            "req4": np.ascontiguousarray(ttr[toks4].reshape(P, 1)),
            "sm4": np.ascontiguousarray(sm[toks4].reshape(P, 1)),
            "ctok": np.ascontiguousarray(toks4.astype(np.int32).reshape(P, 1)),
        }
        for b, (tok0, nb) in enumerate(BATCHES):
            il = p // NRUN            # pair -> local token
            k = p % NRUN              # pair -> run index
            used = p < nb * NRUN
            gtok = np.where(used, c * TPC + tok0 + il, c * TPC)  # pad -> tok 0
            dest = np.where(used, GUT + il * STRIDE + k * BLOCK_SIZE, BIG)
            m[f"preq{b}"] = np.ascontiguousarray(ttr[gtok].reshape(P, 1))
            m[f"ptok{b}"] = np.ascontiguousarray(
                gtok.astype(np.int32).reshape(P, 1))
            m[f"ppk{b}"] = np.ascontiguousarray(
                np.where(used, k, 0).astype(np.int32).reshape(P, 1))
            m[f"ppdest{b}"] = np.ascontiguousarray(
                dest.astype(np.int32).reshape(P, 1))
            m[f"ppsm{b}"] = np.ascontiguousarray(sm[gtok].reshape(P, 1))
        in_maps.append(m)
    return in_maps


def assemble(results):
    swa_indices = np.concatenate(
        [results[c]["o_sidx"].reshape(TPC, WINDOW) for c in range(N_CORES)],
        axis=0)
    swa_lens = np.concatenate(
        [results[c]["o_slen"].reshape(TPC) for c in range(N_CORES)], axis=0)
    pgl = results[0]["o_pgl"].reshape(NUM_PREFILLS)
    gathered = np.empty((NUM_DECODES, WINDOW, HEAD), dtype=np.float32)
    for c in range(N_CORES):
        for b, (tok0, nb) in enumerate(BATCHES):
            arr = results[c][f"o_g{b}"]
            for il in range(nb):
                r0 = GUT + il * STRIDE
                gathered[c * TPC + tok0 + il] = arr[r0:r0 + WINDOW]
    return (swa_indices.astype(np.int32), swa_lens.astype(np.int32),
            pgl.astype(np.int32), gathered)


def kernel(kv_cache, seq_lens, query_start_loc, token_to_req_indices,
           slot_mapping, block_table, window_size=WINDOW, block_size=BLOCK_SIZE,
           num_decodes=NUM_DECODES, num_prefills=NUM_PREFILLS,
           num_decode_tokens=NUM_DECODES, **_ignored):
    assert int(window_size) == WINDOW and int(block_size) == BLOCK_SIZE
    assert int(num_decodes) == NUM_DECODES and int(num_prefills) == NUM_PREFILLS
    nc = get_nc()
    in_maps = make_in_maps(kv_cache, seq_lens, query_start_loc,
                           token_to_req_indices, slot_mapping, block_table)
    res = run_bass_kernel_spmd(nc, in_maps, list(range(N_CORES)))
    _CACHE["last_result"] = res
    return assemble(res.results)
